# revision 35
# baseline (speedup 1.0000x reference)
"""GCN layer (gather -> weighted scatter-sum -> dense transform) on 8 trn2 cores.

Strategy (1-D row partitioning of destination nodes), v2:
  - Core c owns destination nodes [c*NPW, (c+1)*NPW). edge_dst is sorted, so
    each core's edges are a contiguous slice of the edge list.
  - Dst nodes are processed in windows of 128 (PSUM partition size). Each core
    processes ITS windows in DESCENDING edge-count order ("slots"), so the
    shared SPMD per-slot budgets (max over cores) track the per-core sorted
    quantiles and padding stays ~2%. The host unpermutes output columns.
  - Gather indices are int16 for SWDGE dma_gather. Instead of a hard lo/hi
    split at 32768 (which pads two streams separately), we gather from two
    OVERLAPPING views of H: image A = rows [0, 32768), image B = rows
    [17232, 50000) (both int16-addressable). Edges with src in the overlap
    [17232, 32768) are flexible and are assigned per-core to fill image-A
    groups exactly, so each slot needs only ceil(max_edges/128) total groups
    (single rounding).
  - Same-src edge pairs within a window are deduplicated into pair-slots:
    the src row is gathered ONCE and the pair's second edge is applied by an
    extra matmul of the same stationary G tile against a secondary S column
    (pair-slots are packed first in each image section so only a small
    "dup prefix" of groups needs the second matmul). Additionally, srcs
    common to a consecutive pair of slots (2i, 2i+1) are gathered only in
    the even slot; the odd slot applies them via "borrowed" matmuls of the
    even slot's still-resident G tiles against cross S columns that carry
    the odd window's (dst, w). Gather pool bufs=10 keeps two slots' tiles
    alive (bufs=12 crashed the exec unit once -- do not raise). Gathered
    indices: ~190.2k vs 200k raw edges per core.
  - Per 128-slot group: TensorE accumulates aggT[feat, dst] += G.T @ S in
    PSUM, where G = dma_gather'd source rows (fp16) and S is built by two
    DVE tensor_tensor ops in a TRANSPOSED layout [p, dst, group]: every
    operand's innermost AP step is +-1 (drel/wgt broadcast only at the row
    level, a precomputed iota_big constant supplies the dst value), which
    qualifies for the DVE 2x packed perf mode -- halves DVE busy time and
    its SBUF-port contention with the Q7 SWDGE descriptor writes. The
    matmul reads S with a strided moving AP (s[:, :, col]), which costs a
    little extra PE time (idle anyway).
  - The final transform out.T = W.T @ aggT (+b) is interleaved into the
    window loop (one 512-col chunk per 4 finished slots) so it hides under
    the gather stream instead of adding tail latency.
  - dma_gather calls are chunks of <=8 groups (8k+1 SWDGE ring entries;
    k<=8 proven safe on HW; k=11 and k=12 crash the exec unit).
"""

import os
import numpy as np

N_CORES = 8
N_NODES = 50000
D = 128
NPW = N_NODES // N_CORES  # 6250 dst nodes per core
WIN = 128
IMG_A_ROWS = 32768       # image A = H[0:32768]
CUT_B = 32768            # src >= CUT_B must use image B
IMG_B_BASE = N_NODES - 32768  # 17232; image B = H[17232:50000]
# max groups (128 idx each) per dma_gather call. k=8 (65 SWDGE ring
# entries) is proven safe on HW; k=11 and k=12 both CRASH the exec unit
# (tested 2026-08-09) -- do not raise this.
MAXG = int(os.environ.get("GCN_MAXGROUPS", "8"))

GDTYPE = os.environ.get("GCN_GDTYPE", "f16")

LAST_EXEC_NS = None
LAST_RESULTS = None


def _ceil_div(a, b):
    return -(-a // b)


def _dedup_window(s, d, w):
    """Collapse same-src edge pairs within a window into pair-slots.

    Returns slot arrays (src, d1, w1, d2, w2, is_pair): a pair-slot gathers
    its src once and scatters to two dsts (the second via an extra matmul
    against a secondary S column); leftovers become single slots (d2/w2=0).
    """
    o = np.argsort(s, kind="stable")
    ss, dd, ww = s[o], d[o], w[o]
    n = len(ss)
    src_o, d1_o, w1_o, d2_o, w2_o, pair_o = [], [], [], [], [], []
    i = 0
    while i < n:
        j = i
        while j < n and ss[j] == ss[i]:
            j += 1
        k = i
        while k + 1 < j:  # pairs
            src_o.append(ss[i]); d1_o.append(dd[k]); w1_o.append(ww[k])
            d2_o.append(dd[k + 1]); w2_o.append(ww[k + 1]); pair_o.append(True)
            k += 2
        if k < j:  # leftover single
            src_o.append(ss[i]); d1_o.append(dd[k]); w1_o.append(ww[k])
            d2_o.append(0); w2_o.append(0.0); pair_o.append(False)
        i = j
    return (np.array(src_o, np.int64), np.array(d1_o, np.int64),
            np.array(w1_o, np.float64), np.array(d2_o, np.int64),
            np.array(w2_o, np.float64), np.array(pair_o, bool))


def _prep(edge_src, edge_dst, edge_weight):
    """Host-side plan: per-core slot-ordered windows (dedup'd into slots),
    shared slot budgets, dup-group budgets."""
    nwin = _ceil_div(NPW, WIN)
    cores = []
    cnt = np.zeros((N_CORES, nwin), np.int64)
    cntA = np.zeros((N_CORES, nwin), np.int64)  # must-A (src < IMG_B_BASE)
    cntB = np.zeros((N_CORES, nwin), np.int64)  # must-B (src >= CUT_B)
    for c in range(N_CORES):
        e0, e1 = np.searchsorted(edge_dst, [c * NPW, (c + 1) * NPW])
        d = edge_dst[e0:e1] - c * NPW
        s = edge_src[e0:e1]
        w = edge_weight[e0:e1]
        bounds = [np.searchsorted(d, wi * WIN) for wi in range(nwin + 1)]
        wins = []
        for wi in range(nwin):
            i0, i1 = bounds[wi], bounds[wi + 1]
            slots = _dedup_window(s[i0:i1], d[i0:i1] - wi * WIN, w[i0:i1])
            wins.append(slots)
            ssrc = slots[0]
            cnt[c, wi] = len(ssrc)
            cntA[c, wi] = int((ssrc < IMG_B_BASE).sum())
            cntB[c, wi] = int((ssrc >= CUT_B).sum())
        cores.append(wins)

    order = np.argsort(-cnt, axis=1)  # per-core slot -> window
    cs = np.take_along_axis(cnt, order, 1)
    As = np.take_along_axis(cntA, order, 1)
    Bs = np.take_along_axis(cntB, order, 1)

    g_slot = np.maximum(
        _ceil_div(cs.max(0), 128),
        _ceil_div(As.max(0), 128) + _ceil_div(Bs.max(0), 128),
    ).astype(int)
    g_slot = np.maximum(g_slot, 1)
    gA_lo = np.maximum(_ceil_div(As.max(0), 128), 0).astype(int)
    gA_hi = (g_slot - np.maximum(_ceil_div(Bs.max(0), 128), 0)).astype(int)
    # prefer gA a multiple of MAXG (min call count), else low end
    gA_slot = np.empty(nwin, int)
    for i in range(nwin):
        lo, hi = int(gA_lo[i]), int(gA_hi[i])
        lo = max(lo, 0)
        hi = max(hi, lo)
        mult = _ceil_div(lo, MAXG) * MAXG
        gA_slot[i] = mult if lo <= mult <= hi else lo
    gB_slot = g_slot - gA_slot

    # Cross-window sharing: for slot pairs (2i, 2i+1), srcs common to both
    # slots' SINGLE slots are gathered only in the even slot; the odd slot
    # applies them via "borrowed" matmuls of the even slot's G tiles against
    # S columns carrying the odd window's (dst, w).
    packets = [[None] * nwin for _ in range(N_CORES)]
    for c in range(N_CORES):
        for slot in range(nwin):
            src, d1, w1, d2, w2, isp = cores[c][int(order[c][slot])]
            prio = np.where(isp, 0, 2).astype(np.int64)
            packets[c][slot] = [src, d1, w1, d2, w2, prio,
                                np.zeros(len(src), np.int64),
                                np.zeros(len(src), np.float64)]
    for c in range(N_CORES):
        for i in range(nwin // 2):
            t, u = 2 * i, 2 * i + 1
            pt, pu = packets[c][t], packets[c][u]
            st = pt[0][pt[5] == 2]
            su_mask = pu[5] == 2
            su = pu[0][su_mask]
            common = np.intersect1d(st, su)
            if len(common) == 0:
                continue
            tpos = {int(s): k for k, s in enumerate(pt[0])
                    if pt[5][k] == 2}
            upos = {int(pu[0][k]): k for k in np.flatnonzero(su_mask)}
            keep_u = np.ones(len(pu[0]), bool)
            for s in common:
                kt, ku = tpos[int(s)], upos[int(s)]
                pt[5][kt] = 1
                pt[6][kt] = pu[1][ku]
                pt[7][kt] = pu[2][ku]
                keep_u[ku] = False
            for k in range(6):
                pu[k] = pu[k][keep_u]
            pu[6] = pu[6][keep_u]
            pu[7] = pu[7][keep_u]
    for c in range(N_CORES):
        for slot in range(nwin):
            ssrc = packets[c][slot][0]
            cnt[c, slot] = len(ssrc)  # now indexed by SLOT
            cntA[c, slot] = int((ssrc < IMG_B_BASE).sum())
            cntB[c, slot] = int((ssrc >= CUT_B).sum())
    g_slot = np.maximum(
        _ceil_div(cnt.max(0), 128),
        _ceil_div(cntA.max(0), 128) + _ceil_div(cntB.max(0), 128),
    ).astype(int)
    g_slot = np.maximum(g_slot, 1)
    gA_lo = np.maximum(_ceil_div(cntA.max(0), 128), 0).astype(int)
    gA_hi = (g_slot - np.maximum(_ceil_div(cntB.max(0), 128), 0)).astype(int)
    gA_slot = np.empty(nwin, int)
    for i in range(nwin):
        lo = max(int(gA_lo[i]), 0)
        hi = max(int(gA_hi[i]), lo)
        mult = _ceil_div(lo, MAXG) * MAXG
        gA_slot[i] = mult if lo <= mult <= hi else lo
    gB_slot = g_slot - gA_slot

    dupA = np.zeros((N_CORES, nwin), np.int64)
    dupB = np.zeros((N_CORES, nwin), np.int64)
    crsA = np.zeros((N_CORES, nwin), np.int64)
    crsB = np.zeros((N_CORES, nwin), np.int64)
    for c in range(N_CORES):
        for slot in range(nwin):
            src, d1, w1, d2, w2, prio, cd, cw = packets[c][slot]
            selA, selB = _flex_split(src, prio, int(gA_slot[slot]))
            dupA[c, slot] = int((prio[selA] == 0).sum())
            dupB[c, slot] = int((prio[selB] == 0).sum())
            crsA[c, slot] = int((prio[selA] <= 1).sum())
            crsB[c, slot] = int((prio[selB] <= 1).sum())
    dupA_g = np.minimum(_ceil_div(dupA.max(0), 128), gA_slot).astype(int)
    dupB_g = np.minimum(_ceil_div(dupB.max(0), 128), gB_slot).astype(int)
    crsA_g = np.zeros(nwin, int)
    crsB_g = np.zeros(nwin, int)
    for i in range(nwin // 2):
        t = 2 * i
        crsA_g[t] = min(int(_ceil_div(crsA[:, t].max(), 128)), int(gA_slot[t]))
        crsB_g[t] = min(int(_ceil_div(crsB[:, t].max(), 128)), int(gB_slot[t]))
    return (packets, order, nwin, g_slot, gA_slot, gB_slot, dupA_g, dupB_g,
            crsA_g, crsB_g)


def _flex_split(src, prio, gA):
    """Assign slots to image sections: forced by src range, flex fills A up
    to capacity. Within each section slots are ordered by priority class
    (0=pair first, 1=cross-shared, 2=plain). Returns (selA, selB)."""
    isA_forced = src < IMG_B_BASE
    isB_forced = src >= CUT_B
    flex = ~isA_forced & ~isB_forced
    capA = 128 * gA
    nA0 = int(isA_forced.sum())
    take = min(max(capA - nA0, 0), int(flex.sum()))
    idxF = np.flatnonzero(flex)
    selA = np.concatenate([np.flatnonzero(isA_forced), idxF[:take]])
    selB = np.concatenate([idxF[take:], np.flatnonzero(isB_forced)])
    selA = selA[np.argsort(prio[selA], kind="stable")]
    selB = selB[np.argsort(prio[selB], kind="stable")]
    return selA.astype(np.int64), selB.astype(np.int64)


def _chunks(g):
    out = []
    c0 = 0
    while c0 < g:
        k = min(MAXG, g - c0)
        out.append((c0, k))
        c0 += k
    return out


def _pack_core(packets_c, nwin, g_slot, gA_slot, gB_slot, dupA_g, dupB_g,
               crsA_g, crsB_g, np_g):
    """Build device arrays for one core.

    idx: per-call wrapped-16 blocks, concatenated, tiled to [128, .].
    drel/wgt: [128, sum(g + dupA_g + dupB_g)]: per window, columns are
        [g primary][dupA_g secondary for groups 0..][dupB_g secondary for
        groups gA..]; row p = slot at group position p."""
    np_m = np_g
    tot_cols = int((g_slot + dupA_g + dupB_g + crsA_g + crsB_g).sum())
    drel = np.zeros((128, tot_cols), np_m)
    wgt = np.zeros((128, tot_cols), np_m)
    idx_blocks = []
    cbase = 0
    for slot in range(nwin):
        src, d1, w1, d2, w2, prio, cd, cw = packets_c[slot]
        gA, gB, g = int(gA_slot[slot]), int(gB_slot[slot]), int(g_slot[slot])
        dA, dB = int(dupA_g[slot]), int(dupB_g[slot])
        cA, cB = int(crsA_g[slot]), int(crsB_g[slot])
        selA, selB = _flex_split(src, prio, gA)
        assert len(selA) <= 128 * gA and len(selB) <= 128 * gB, (
            slot, len(selA), gA, len(selB), gB)
        iA = np.zeros(128 * gA, np.int16)
        iB = np.zeros(128 * gB, np.int16)
        iA[: len(selA)] = src[selA].astype(np.int16)
        iB[: len(selB)] = (src[selB] - IMG_B_BASE).astype(np.int16)
        # primary metadata per slot position (A section then B section)
        dd = np.zeros(128 * g, np_m)
        ww = np.zeros(128 * g, np_m)
        dd[: len(selA)] = d1[selA].astype(np_m)
        ww[: len(selA)] = w1[selA].astype(np_g).astype(np_m)
        dd[128 * gA : 128 * gA + len(selB)] = d1[selB].astype(np_m)
        ww[128 * gA : 128 * gA + len(selB)] = w1[selB].astype(np_g).astype(np_m)
        drel[:, cbase : cbase + g] = dd.reshape(g, 128).T
        wgt[:, cbase : cbase + g] = ww.reshape(g, 128).T
        # secondary metadata for dup-prefix groups (zeros for singles)
        dd2 = np.zeros(128 * (dA + dB), np_m)
        ww2 = np.zeros(128 * (dA + dB), np_m)
        nA2 = min(len(selA), 128 * dA)
        dd2[:nA2] = d2[selA[:nA2]].astype(np_m)
        ww2[:nA2] = w2[selA[:nA2]].astype(np_g).astype(np_m)
        nB2 = min(len(selB), 128 * dB)
        dd2[128 * dA : 128 * dA + nB2] = d2[selB[:nB2]].astype(np_m)
        ww2[128 * dA : 128 * dA + nB2] = w2[selB[:nB2]].astype(np_g).astype(np_m)
        if dA + dB:
            drel[:, cbase + g : cbase + g + dA + dB] = (
                dd2.reshape(dA + dB, 128).T)
            wgt[:, cbase + g : cbase + g + dA + dB] = (
                ww2.reshape(dA + dB, 128).T)
        # cross (borrowed) columns: odd window's (dst, w) at prio-1 slot
        # positions of the FIRST cA/cB groups of each section, zeros else
        if cA + cB:
            dd3 = np.zeros(128 * (cA + cB), np_m)
            ww3 = np.zeros(128 * (cA + cB), np_m)
            posA = np.flatnonzero(prio[selA] == 1)
            assert posA.size == 0 or posA.max() < 128 * cA
            dd3[posA] = cd[selA[posA]].astype(np_m)
            ww3[posA] = cw[selA[posA]].astype(np_g).astype(np_m)
            posB = np.flatnonzero(prio[selB] == 1)
            assert posB.size == 0 or posB.max() < 128 * cB
            dd3[128 * cA + posB] = cd[selB[posB]].astype(np_m)
            ww3[128 * cA + posB] = cw[selB[posB]].astype(np_g).astype(np_m)
            c0x = cbase + g + dA + dB
            drel[:, c0x : c0x + cA + cB] = dd3.reshape(cA + cB, 128).T
            wgt[:, c0x : c0x + cA + cB] = ww3.reshape(cA + cB, 128).T
        # idx blocks per call (A calls then B calls), wrapped 16
        for (c0, k) in _chunks(gA):
            idx_blocks.append(iA[c0 * 128 : (c0 + k) * 128].reshape(-1, 16).T)
        for (c0, k) in _chunks(gB):
            idx_blocks.append(iB[c0 * 128 : (c0 + k) * 128].reshape(-1, 16).T)
        cbase += g + dA + dB + cA + cB
    idx = np.tile(np.concatenate(idx_blocks, axis=1), (8, 1))
    return idx, np.ascontiguousarray(drel), np.ascontiguousarray(wgt)


def _build_program(nwin, g_slot, gA_slot, gB_slot, dupA_g, dupB_g, crsA_g,
                   crsB_g, idx_cols, n_cores=N_CORES):
    from contextlib import ExitStack

    import concourse.tile as tile
    from concourse import bacc, mybir

    f32 = mybir.dt.float32
    gdt = mybir.dt.float16 if GDTYPE == "f16" else mybir.dt.float32
    i16 = mybir.dt.int16

    nc = bacc.Bacc(
        "TRN2", target_bir_lowering=False, debug=False, num_devices=n_cores,
    )

    npad = nwin * WIN
    tot_g = int((g_slot + dupA_g + dupB_g + crsA_g + crsB_g).sum())

    h_t = nc.dram_tensor("h_src", [N_NODES, D], gdt, kind="ExternalInput")
    idx_t = nc.dram_tensor("idx", [128, idx_cols], i16, kind="ExternalInput")
    drel_t = nc.dram_tensor("drel", [128, tot_g], gdt, kind="ExternalInput")
    wgt_t = nc.dram_tensor("wgt", [128, tot_g], gdt, kind="ExternalInput")
    gmx = int((g_slot + dupA_g + dupB_g + crsA_g + crsB_g).max())
    iota_t = nc.dram_tensor("iota", [128, 128 * gmx], gdt, kind="ExternalInput")
    w_t = nc.dram_tensor("wmat", [D, D], gdt, kind="ExternalInput")
    b_t = nc.dram_tensor("bcol", [D, 1], f32, kind="ExternalInput")
    out_t = nc.dram_tensor("outT", [D, npad], f32, kind="ExternalOutput")

    with tile.TileContext(nc) as tc:
        with ExitStack() as ctx:
            const = ctx.enter_context(tc.tile_pool(name="const", bufs=1))
            gpool = ctx.enter_context(tc.tile_pool(name="gather", bufs=10))
            spool = ctx.enter_context(tc.tile_pool(name="sel", bufs=3))
            opool = ctx.enter_context(tc.tile_pool(name="outsb", bufs=2))
            ps_agg = ctx.enter_context(tc.tile_pool(name="ps_agg", bufs=2, space="PSUM"))
            ps_out = ctx.enter_context(tc.tile_pool(name="ps_out", bufs=2, space="PSUM"))

            idx = const.tile(list(idx_t.shape), i16)
            drel = const.tile(list(drel_t.shape), gdt)
            wgt = const.tile(list(wgt_t.shape), gdt)
            iota = const.tile([128, 128, gmx], gdt)
            wmat = const.tile([D, D], gdt)
            bcol = const.tile([D, 1], f32)
            agg_all = const.tile([128, npad], gdt, tag="agg_all")

            for sb, dr in ((idx, idx_t), (drel, drel_t), (wgt, wgt_t),
                           (iota, iota_t), (wmat, w_t), (bcol, b_t)):
                nc.sync.dma_start(sb[:], dr[:])

            h_A = h_t[0:IMG_A_ROWS, :]
            h_B = h_t[IMG_B_BASE:N_NODES, :]

            col = 0    # idx column cursor (units of 8 cols per group)
            gbase = 0  # group column cursor
            done_slots = 0
            next_t0 = 0
            CH = 512

            def emit_transform(t0, n):
                po = ps_out.tile([128, CH], f32, tag="psout")
                nc.tensor.matmul(
                    po[:, :n], wmat[:], agg_all[:, t0 : t0 + n],
                    start=True, stop=True,
                )
                ob = opool.tile([128, CH], f32, tag="outsb")
                nc.scalar.add(ob[:, :n], po[:, :n], bcol[:])
                nc.sync.dma_start(out_t[:, t0 : t0 + n], ob[:, :n])

            prev = None  # (group_tiles, s, cross_col0, cA, cB, gA)
            for slot in range(nwin):
                gA, gB, g = int(gA_slot[slot]), int(gB_slot[slot]), int(g_slot[slot])
                dA, dB = int(dupA_g[slot]), int(dupB_g[slot])
                cA, cB = int(crsA_g[slot]), int(crsB_g[slot])
                ncols = g + dA + dB + cA + cB
                gtiles = []
                group_tiles = []  # flat (tile, j) per group, A then B
                for img, gimg in ((h_A, gA), (h_B, gB)):
                    for (c0, k) in _chunks(gimg):
                        gt = gpool.tile([128, k, 128], gdt, tag="g")
                        nc.gpsimd.dma_gather(
                            gt[:], img, idx[:, col : col + k * 8],
                            num_idxs=k * 128, num_idxs_reg=k * 128, elem_size=D,
                        )
                        col += k * 8
                        gtiles.append((gt, k))
                        for j in range(k):
                            group_tiles.append((gt, j))

                # S transposed [p, dst, group]: innermost AP step is 1 on
                # every operand (drel/wgt broadcast at the ROW level only),
                # which qualifies for the DVE 2x packed perf mode; the
                # iota_big constant supplies value n for all of row n.
                s = spool.tile([128, 128, ncols], gdt, tag="sel")
                sh = (128, 128, ncols)
                if os.environ.get("GCN_SPROBE", "0") == "1":
                    # perf probe: no DVE S-build (output is wrong)
                    nc.vector.memset(s[:], 0)
                else:
                    nc.vector.tensor_tensor(
                        s[:], iota[:, :, 0:ncols],
                        drel[:, None, gbase : gbase + ncols].broadcast_to(sh),
                        mybir.AluOpType.is_equal,
                    )
                    nc.vector.tensor_tensor(
                        s[:], s[:],
                        wgt[:, None, gbase : gbase + ncols].broadcast_to(sh),
                        mybir.AluOpType.mult,
                    )

                # matmul plan: group j -> primary S col j; dup-prefix groups
                # also get a secondary matmul (same stationary G).
                plan = []
                for j in range(g):
                    cols_j = [j]
                    if j < dA:
                        cols_j.append(g + j)
                    elif gA <= j < gA + dB:
                        cols_j.append(g + dA + (j - gA))
                    plan.append(cols_j)
                n_mm = sum(len(cj) for cj in plan)
                borrowed = []
                if slot % 2 == 1 and prev is not None:
                    pgt, ps_, pc0, pcA, pcB, pgA = prev
                    for j in range(pcA):
                        borrowed.append((pgt[j], ps_, pc0 + j))
                    for j in range(pcB):
                        borrowed.append((pgt[pgA + j], ps_, pc0 + pcA + j))
                n_mm += len(borrowed)

                psum = ps_agg.tile([128, 128], f32, tag="psagg")
                mm = 0
                for ((bgt, bj), bs, bcol_) in borrowed:
                    nc.tensor.matmul(
                        psum[:], bgt[:, bj, :], bs[:, :, bcol_],
                        start=(mm == 0), stop=(mm == n_mm - 1),
                    )
                    mm += 1
                gi = 0
                for (gt, k) in gtiles:
                    for j in range(k):
                        for scol in plan[gi]:
                            nc.tensor.matmul(
                                psum[:], gt[:, j, :], s[:, :, scol],
                                start=(mm == 0), stop=(mm == n_mm - 1),
                            )
                            mm += 1
                        gi += 1
                nc.scalar.copy(agg_all[:, slot * WIN : (slot + 1) * WIN], psum[:])
                prev = (group_tiles, s, g + dA + dB, cA, cB, gA)
                gbase += ncols
                done_slots += 1
                # transform any complete 512-col chunk whose slots are done
                while done_slots * WIN >= next_t0 + CH:
                    emit_transform(next_t0, CH)
                    next_t0 += CH

            while next_t0 < npad:
                n = min(CH, npad - next_t0)
                emit_transform(next_t0, n)
                next_t0 += n

    nc.compile()
    return nc


def kernel(H, edge_src, edge_dst, edge_weight, W, b):
    global LAST_EXEC_NS, LAST_RESULTS
    from concourse import bass_utils

    H = np.asarray(H, dtype=np.float32)
    edge_src = np.asarray(edge_src, dtype=np.int32)
    edge_dst = np.asarray(edge_dst, dtype=np.int32)
    edge_weight = np.asarray(edge_weight, dtype=np.float32)
    W = np.asarray(W, dtype=np.float32)
    b = np.asarray(b, dtype=np.float32)

    np_g = np.float16 if GDTYPE == "f16" else np.float32
    (packets, order, nwin, g_slot, gA_slot, gB_slot, dupA_g, dupB_g,
     crsA_g, crsB_g) = _prep(edge_src, edge_dst, edge_weight)

    h_src = np.ascontiguousarray(H.astype(np_g))
    gmx = int((g_slot + dupA_g + dupB_g + crsA_g + crsB_g).max())
    iota = np.tile(np.repeat(np.arange(128, dtype=np_g), gmx), (128, 1))
    wmat = np.ascontiguousarray(W.astype(np_g))
    bcol = np.ascontiguousarray(b.astype(np.float32).reshape(D, 1))
    in_maps = []
    idx_cols = None
    for c in range(N_CORES):
        idx, drel, wgt = _pack_core(
            packets[c], nwin, g_slot, gA_slot, gB_slot, dupA_g, dupB_g,
            crsA_g, crsB_g, np_g,
        )
        idx_cols = idx.shape[1]
        in_maps.append({
            "h_src": h_src, "idx": idx, "drel": drel, "wgt": wgt,
            "iota": iota, "wmat": wmat, "bcol": bcol,
        })

    nc = _build_program(nwin, g_slot, gA_slot, gB_slot, dupA_g, dupB_g,
                        crsA_g, crsB_g, idx_cols)

    if os.environ.get("GCN_SIM", "0") == "1":
        from concourse.bass_interp import CoreSim

        out = np.empty((N_NODES, D), np.float32)
        for c in range(N_CORES):
            sim = CoreSim(nc)
            for k2, v2 in in_maps[c].items():
                sim.tensor(k2)[:] = v2
            sim.simulate()
            outT = np.array(sim.tensor("outT"))
            # slot i columns -> window order[c][i]
            for slot in range(nwin):
                wi = int(order[c][slot])
                w0, w1 = wi * WIN, min((wi + 1) * WIN, NPW)
                out[c * NPW + w0 : c * NPW + w1, :] = (
                    outT[:, slot * WIN : slot * WIN + (w1 - w0)].T)
        return out

    trace = os.environ.get("GCN_TRACE", "0") == "1"
    kw = {}
    if trace:
        import shutil
        td = "/tmp/gcn_ntff"
        shutil.rmtree(td, ignore_errors=True)
        os.makedirs(td, exist_ok=True)
        kw["tmpdir"] = td
    import time as _time
    last_err = None
    for backoff in (15, 45, 90, 0):
        try:
            res = bass_utils.run_bass_kernel_spmd(
                nc, in_maps, core_ids=list(range(N_CORES)), trace=trace, **kw
            )
            break
        except Exception as e:
            last_err = e
            if backoff:
                _time.sleep(backoff)
    else:
        raise last_err
    LAST_EXEC_NS = res.exec_time_ns
    LAST_RESULTS = res

    out = np.empty((N_NODES, D), np.float32)
    for c in range(N_CORES):
        outT = res.results[c]["outT"]
        for slot in range(nwin):
            wi = int(order[c][slot])
            w0, w1 = wi * WIN, min((wi + 1) * WIN, NPW)
            out[c * NPW + w0 : c * NPW + w1, :] = (
                outT[:, slot * WIN : slot * WIN + (w1 - w0)].T)
    return out


# revision 36
# speedup vs baseline: 1.2343x; 1.2343x over previous
"""GCN layer (gather -> weighted scatter-sum -> dense transform) on 8 trn2 cores.

Strategy (1-D row partitioning of destination nodes), v2:
  - Core c owns destination nodes [c*NPW, (c+1)*NPW). edge_dst is sorted, so
    each core's edges are a contiguous slice of the edge list.
  - Dst nodes are processed in windows of 128 (PSUM partition size). Each core
    processes ITS windows in DESCENDING edge-count order ("slots"), so the
    shared SPMD per-slot budgets (max over cores) track the per-core sorted
    quantiles and padding stays ~2%. The host unpermutes output columns.
  - Gather indices are int16 for SWDGE dma_gather. Instead of a hard lo/hi
    split at 32768 (which pads two streams separately), we gather from two
    OVERLAPPING views of H: image A = rows [0, 32768), image B = rows
    [17232, 50000) (both int16-addressable). Edges with src in the overlap
    [17232, 32768) are flexible and are assigned per-core to fill image-A
    groups exactly, so each slot needs only ceil(max_edges/128) total groups
    (single rounding).
  - Same-src edge pairs within a window are deduplicated into pair-slots:
    the src row is gathered ONCE and the pair's second edge is applied by an
    extra matmul of the same stationary G tile against a secondary S column
    (pair-slots are packed first in each image section so only a small
    "dup prefix" of groups needs the second matmul). Additionally, srcs
    common to a consecutive pair of slots (2i, 2i+1) are gathered only in
    the even slot; the odd slot applies them via "borrowed" matmuls of the
    even slot's still-resident G tiles against cross S columns that carry
    the odd window's (dst, w). Gather pool bufs=10 keeps two slots' tiles
    alive (bufs=12 crashed the exec unit once -- do not raise). Gathered
    indices: ~190.2k vs 200k raw edges per core.
  - Per 128-slot group: TensorE accumulates aggT[feat, dst] += G.T @ S in
    PSUM, where G = dma_gather'd source rows (fp16) and S is built by two
    DVE tensor_tensor ops in a TRANSPOSED layout [p, dst, group]: every
    operand's innermost AP step is +-1 (drel/wgt broadcast only at the row
    level, a precomputed iota_big constant supplies the dst value), which
    qualifies for the DVE 2x packed perf mode -- halves DVE busy time and
    its SBUF-port contention with the Q7 SWDGE descriptor writes. The
    matmul reads S with a strided moving AP (s[:, :, col]), which costs a
    little extra PE time (idle anyway).
  - The final transform out.T = W.T @ aggT (+b) is interleaved into the
    window loop (one 512-col chunk per 4 finished slots) so it hides under
    the gather stream instead of adding tail latency.
  - dma_gather calls are chunks of <=8 groups (8k+1 SWDGE ring entries;
    k<=8 proven safe on HW; k=11 and k=12 crash the exec unit).
"""

import os
import numpy as np

N_CORES = 8
N_NODES = 50000
D = 128
NPW = N_NODES // N_CORES  # 6250 dst nodes per core
WIN = 128
IMG_A_ROWS = 32768       # image A = H[0:32768]
CUT_B = 32768            # src >= CUT_B must use image B
IMG_B_BASE = N_NODES - 32768  # 17232; image B = H[17232:50000]
# max groups (128 idx each) per dma_gather call. k=8 (65 SWDGE ring
# entries) is proven safe on HW; k=11 and k=12 both CRASH the exec unit
# (tested 2026-08-09) -- do not raise this.
MAXG = int(os.environ.get("GCN_MAXGROUPS", "8"))

GDTYPE = os.environ.get("GCN_GDTYPE", "f16")

LAST_EXEC_NS = None
LAST_RESULTS = None


def _ceil_div(a, b):
    return -(-a // b)


def _dedup_window(s, d, w):
    """Collapse same-src edge pairs within a window into pair-slots.

    Returns slot arrays (src, d1, w1, d2, w2, is_pair): a pair-slot gathers
    its src once and scatters to two dsts (the second via an extra matmul
    against a secondary S column); leftovers become single slots (d2/w2=0).
    """
    o = np.argsort(s, kind="stable")
    ss, dd, ww = s[o], d[o], w[o]
    n = len(ss)
    src_o, d1_o, w1_o, d2_o, w2_o, pair_o = [], [], [], [], [], []
    i = 0
    while i < n:
        j = i
        while j < n and ss[j] == ss[i]:
            j += 1
        k = i
        while k + 1 < j:  # pairs
            src_o.append(ss[i]); d1_o.append(dd[k]); w1_o.append(ww[k])
            d2_o.append(dd[k + 1]); w2_o.append(ww[k + 1]); pair_o.append(True)
            k += 2
        if k < j:  # leftover single
            src_o.append(ss[i]); d1_o.append(dd[k]); w1_o.append(ww[k])
            d2_o.append(0); w2_o.append(0.0); pair_o.append(False)
        i = j
    return (np.array(src_o, np.int64), np.array(d1_o, np.int64),
            np.array(w1_o, np.float64), np.array(d2_o, np.int64),
            np.array(w2_o, np.float64), np.array(pair_o, bool))


def _prep(edge_src, edge_dst, edge_weight):
    """Host-side plan: per-core slot-ordered windows (dedup'd into slots),
    shared slot budgets, dup-group budgets."""
    nwin = _ceil_div(NPW, WIN)
    cores = []
    cnt = np.zeros((N_CORES, nwin), np.int64)
    cntA = np.zeros((N_CORES, nwin), np.int64)  # must-A (src < IMG_B_BASE)
    cntB = np.zeros((N_CORES, nwin), np.int64)  # must-B (src >= CUT_B)
    for c in range(N_CORES):
        e0, e1 = np.searchsorted(edge_dst, [c * NPW, (c + 1) * NPW])
        d = edge_dst[e0:e1] - c * NPW
        s = edge_src[e0:e1]
        w = edge_weight[e0:e1]
        bounds = [np.searchsorted(d, wi * WIN) for wi in range(nwin + 1)]
        wins = []
        for wi in range(nwin):
            i0, i1 = bounds[wi], bounds[wi + 1]
            slots = _dedup_window(s[i0:i1], d[i0:i1] - wi * WIN, w[i0:i1])
            wins.append(slots)
            ssrc = slots[0]
            cnt[c, wi] = len(ssrc)
            cntA[c, wi] = int((ssrc < IMG_B_BASE).sum())
            cntB[c, wi] = int((ssrc >= CUT_B).sum())
        cores.append(wins)

    order = np.argsort(-cnt, axis=1)  # per-core slot -> window
    cs = np.take_along_axis(cnt, order, 1)
    As = np.take_along_axis(cntA, order, 1)
    Bs = np.take_along_axis(cntB, order, 1)

    g_slot = np.maximum(
        _ceil_div(cs.max(0), 128),
        _ceil_div(As.max(0), 128) + _ceil_div(Bs.max(0), 128),
    ).astype(int)
    g_slot = np.maximum(g_slot, 1)
    gA_lo = np.maximum(_ceil_div(As.max(0), 128), 0).astype(int)
    gA_hi = (g_slot - np.maximum(_ceil_div(Bs.max(0), 128), 0)).astype(int)
    # prefer gA a multiple of MAXG (min call count), else low end
    gA_slot = np.empty(nwin, int)
    for i in range(nwin):
        lo, hi = int(gA_lo[i]), int(gA_hi[i])
        lo = max(lo, 0)
        hi = max(hi, lo)
        mult = _ceil_div(lo, MAXG) * MAXG
        gA_slot[i] = mult if lo <= mult <= hi else lo
    gB_slot = g_slot - gA_slot

    # Cross-window sharing: for slot pairs (2i, 2i+1), srcs common to both
    # slots' SINGLE slots are gathered only in the even slot; the odd slot
    # applies them via "borrowed" matmuls of the even slot's G tiles against
    # S columns carrying the odd window's (dst, w).
    packets = [[None] * nwin for _ in range(N_CORES)]
    for c in range(N_CORES):
        for slot in range(nwin):
            src, d1, w1, d2, w2, isp = cores[c][int(order[c][slot])]
            prio = np.where(isp, 0, 2).astype(np.int64)
            packets[c][slot] = [src, d1, w1, d2, w2, prio,
                                np.zeros(len(src), np.int64),
                                np.zeros(len(src), np.float64)]
    for c in range(N_CORES):
        for t in range(nwin - 1):
            u = t + 1
            pt, pu = packets[c][t], packets[c][u]
            st = pt[0][pt[5] == 2]
            su_mask = pu[5] == 2
            su = pu[0][su_mask]
            common = np.intersect1d(st, su)
            if len(common) == 0:
                continue
            tpos = {int(s): k for k, s in enumerate(pt[0])
                    if pt[5][k] == 2}
            upos = {int(pu[0][k]): k for k in np.flatnonzero(su_mask)}
            keep_u = np.ones(len(pu[0]), bool)
            for s in common:
                kt, ku = tpos[int(s)], upos[int(s)]
                pt[5][kt] = 1
                pt[6][kt] = pu[1][ku]
                pt[7][kt] = pu[2][ku]
                keep_u[ku] = False
            for k in range(6):
                pu[k] = pu[k][keep_u]
            pu[6] = pu[6][keep_u]
            pu[7] = pu[7][keep_u]
    for c in range(N_CORES):
        for slot in range(nwin):
            ssrc = packets[c][slot][0]
            cnt[c, slot] = len(ssrc)  # now indexed by SLOT
            cntA[c, slot] = int((ssrc < IMG_B_BASE).sum())
            cntB[c, slot] = int((ssrc >= CUT_B).sum())
    g_slot = np.maximum(
        _ceil_div(cnt.max(0), 128),
        _ceil_div(cntA.max(0), 128) + _ceil_div(cntB.max(0), 128),
    ).astype(int)
    g_slot = np.maximum(g_slot, 1)
    gA_lo = np.maximum(_ceil_div(cntA.max(0), 128), 0).astype(int)
    gA_hi = (g_slot - np.maximum(_ceil_div(cntB.max(0), 128), 0)).astype(int)
    gA_slot = np.empty(nwin, int)
    for i in range(nwin):
        lo = max(int(gA_lo[i]), 0)
        hi = max(int(gA_hi[i]), lo)
        mult = _ceil_div(lo, MAXG) * MAXG
        gA_slot[i] = mult if lo <= mult <= hi else lo
    gB_slot = g_slot - gA_slot

    dupA = np.zeros((N_CORES, nwin), np.int64)
    dupB = np.zeros((N_CORES, nwin), np.int64)
    crsA = np.zeros((N_CORES, nwin), np.int64)
    crsB = np.zeros((N_CORES, nwin), np.int64)
    for c in range(N_CORES):
        for slot in range(nwin):
            src, d1, w1, d2, w2, prio, cd, cw = packets[c][slot]
            selA, selB = _flex_split(src, prio, int(gA_slot[slot]))
            dupA[c, slot] = int((prio[selA] == 0).sum())
            dupB[c, slot] = int((prio[selB] == 0).sum())
            crsA[c, slot] = int((prio[selA] <= 1).sum())
            crsB[c, slot] = int((prio[selB] <= 1).sum())
    dupA_g = np.minimum(_ceil_div(dupA.max(0), 128), gA_slot).astype(int)
    dupB_g = np.minimum(_ceil_div(dupB.max(0), 128), gB_slot).astype(int)
    crsA_g = np.zeros(nwin, int)
    crsB_g = np.zeros(nwin, int)
    for t in range(nwin - 1):
        crsA_g[t] = min(int(_ceil_div(crsA[:, t].max(), 128)), int(gA_slot[t]))
        crsB_g[t] = min(int(_ceil_div(crsB[:, t].max(), 128)), int(gB_slot[t]))
    return (packets, order, nwin, g_slot, gA_slot, gB_slot, dupA_g, dupB_g,
            crsA_g, crsB_g)


def _flex_split(src, prio, gA):
    """Assign slots to image sections: forced by src range, flex fills A up
    to capacity. Within each section slots are ordered by priority class
    (0=pair first, 1=cross-shared, 2=plain). Returns (selA, selB)."""
    isA_forced = src < IMG_B_BASE
    isB_forced = src >= CUT_B
    flex = ~isA_forced & ~isB_forced
    capA = 128 * gA
    nA0 = int(isA_forced.sum())
    take = min(max(capA - nA0, 0), int(flex.sum()))
    idxF = np.flatnonzero(flex)
    selA = np.concatenate([np.flatnonzero(isA_forced), idxF[:take]])
    selB = np.concatenate([idxF[take:], np.flatnonzero(isB_forced)])
    selA = selA[np.argsort(prio[selA], kind="stable")]
    selB = selB[np.argsort(prio[selB], kind="stable")]
    return selA.astype(np.int64), selB.astype(np.int64)


def _chunks(g):
    out = []
    c0 = 0
    while c0 < g:
        k = min(MAXG, g - c0)
        out.append((c0, k))
        c0 += k
    return out


def _pack_core(packets_c, nwin, g_slot, gA_slot, gB_slot, dupA_g, dupB_g,
               crsA_g, crsB_g, np_g):
    """Build device arrays for one core.

    idx: per-call wrapped-16 blocks, concatenated, tiled to [128, .].
    drel/wgt: [128, sum(g + dupA_g + dupB_g)]: per window, columns are
        [g primary][dupA_g secondary for groups 0..][dupB_g secondary for
        groups gA..]; row p = slot at group position p."""
    np_m = np_g
    tot_cols = int((g_slot + dupA_g + dupB_g + crsA_g + crsB_g).sum())
    drel = np.zeros((128, tot_cols), np_m)
    wgt = np.zeros((128, tot_cols), np_m)
    idx_blocks = []
    cbase = 0
    for slot in range(nwin):
        src, d1, w1, d2, w2, prio, cd, cw = packets_c[slot]
        gA, gB, g = int(gA_slot[slot]), int(gB_slot[slot]), int(g_slot[slot])
        dA, dB = int(dupA_g[slot]), int(dupB_g[slot])
        cA, cB = int(crsA_g[slot]), int(crsB_g[slot])
        selA, selB = _flex_split(src, prio, gA)
        assert len(selA) <= 128 * gA and len(selB) <= 128 * gB, (
            slot, len(selA), gA, len(selB), gB)
        iA = np.zeros(128 * gA, np.int16)
        iB = np.zeros(128 * gB, np.int16)
        iA[: len(selA)] = src[selA].astype(np.int16)
        iB[: len(selB)] = (src[selB] - IMG_B_BASE).astype(np.int16)
        # primary metadata per slot position (A section then B section)
        dd = np.zeros(128 * g, np_m)
        ww = np.zeros(128 * g, np_m)
        dd[: len(selA)] = d1[selA].astype(np_m)
        ww[: len(selA)] = w1[selA].astype(np_g).astype(np_m)
        dd[128 * gA : 128 * gA + len(selB)] = d1[selB].astype(np_m)
        ww[128 * gA : 128 * gA + len(selB)] = w1[selB].astype(np_g).astype(np_m)
        drel[:, cbase : cbase + g] = dd.reshape(g, 128).T
        wgt[:, cbase : cbase + g] = ww.reshape(g, 128).T
        # secondary metadata for dup-prefix groups (zeros for singles)
        dd2 = np.zeros(128 * (dA + dB), np_m)
        ww2 = np.zeros(128 * (dA + dB), np_m)
        nA2 = min(len(selA), 128 * dA)
        dd2[:nA2] = d2[selA[:nA2]].astype(np_m)
        ww2[:nA2] = w2[selA[:nA2]].astype(np_g).astype(np_m)
        nB2 = min(len(selB), 128 * dB)
        dd2[128 * dA : 128 * dA + nB2] = d2[selB[:nB2]].astype(np_m)
        ww2[128 * dA : 128 * dA + nB2] = w2[selB[:nB2]].astype(np_g).astype(np_m)
        if dA + dB:
            drel[:, cbase + g : cbase + g + dA + dB] = (
                dd2.reshape(dA + dB, 128).T)
            wgt[:, cbase + g : cbase + g + dA + dB] = (
                ww2.reshape(dA + dB, 128).T)
        # cross (borrowed) columns: odd window's (dst, w) at prio-1 slot
        # positions of the FIRST cA/cB groups of each section, zeros else
        if cA + cB:
            dd3 = np.zeros(128 * (cA + cB), np_m)
            ww3 = np.zeros(128 * (cA + cB), np_m)
            posA = np.flatnonzero(prio[selA] == 1)
            assert posA.size == 0 or posA.max() < 128 * cA
            dd3[posA] = cd[selA[posA]].astype(np_m)
            ww3[posA] = cw[selA[posA]].astype(np_g).astype(np_m)
            posB = np.flatnonzero(prio[selB] == 1)
            assert posB.size == 0 or posB.max() < 128 * cB
            dd3[128 * cA + posB] = cd[selB[posB]].astype(np_m)
            ww3[128 * cA + posB] = cw[selB[posB]].astype(np_g).astype(np_m)
            c0x = cbase + g + dA + dB
            drel[:, c0x : c0x + cA + cB] = dd3.reshape(cA + cB, 128).T
            wgt[:, c0x : c0x + cA + cB] = ww3.reshape(cA + cB, 128).T
        # idx blocks per call (A calls then B calls), wrapped 16
        for (c0, k) in _chunks(gA):
            idx_blocks.append(iA[c0 * 128 : (c0 + k) * 128].reshape(-1, 16).T)
        for (c0, k) in _chunks(gB):
            idx_blocks.append(iB[c0 * 128 : (c0 + k) * 128].reshape(-1, 16).T)
        cbase += g + dA + dB + cA + cB
    idx = np.tile(np.concatenate(idx_blocks, axis=1), (8, 1))
    return idx, np.ascontiguousarray(drel), np.ascontiguousarray(wgt)


def _build_program(nwin, g_slot, gA_slot, gB_slot, dupA_g, dupB_g, crsA_g,
                   crsB_g, idx_cols, n_cores=N_CORES):
    from contextlib import ExitStack

    import concourse.tile as tile
    from concourse import bacc, mybir

    f32 = mybir.dt.float32
    gdt = mybir.dt.float16 if GDTYPE == "f16" else mybir.dt.float32
    i16 = mybir.dt.int16

    nc = bacc.Bacc(
        "TRN2", target_bir_lowering=False, debug=False, num_devices=n_cores,
    )

    npad = nwin * WIN
    tot_g = int((g_slot + dupA_g + dupB_g + crsA_g + crsB_g).sum())

    h_t = nc.dram_tensor("h_src", [N_NODES, D], gdt, kind="ExternalInput")
    idx_t = nc.dram_tensor("idx", [128, idx_cols], i16, kind="ExternalInput")
    drel_t = nc.dram_tensor("drel", [128, tot_g], gdt, kind="ExternalInput")
    wgt_t = nc.dram_tensor("wgt", [128, tot_g], gdt, kind="ExternalInput")
    gmx = int((g_slot + dupA_g + dupB_g + crsA_g + crsB_g).max())
    iota_t = nc.dram_tensor("iota", [128, 128 * gmx], gdt, kind="ExternalInput")
    w_t = nc.dram_tensor("wmat", [D, D], gdt, kind="ExternalInput")
    b_t = nc.dram_tensor("bcol", [D, 1], f32, kind="ExternalInput")
    out_t = nc.dram_tensor("outT", [D, npad], f32, kind="ExternalOutput")

    with tile.TileContext(nc) as tc:
        with ExitStack() as ctx:
            const = ctx.enter_context(tc.tile_pool(name="const", bufs=1))
            gpool = ctx.enter_context(tc.tile_pool(name="gather", bufs=10))
            spool = ctx.enter_context(tc.tile_pool(name="sel", bufs=3))
            opool = ctx.enter_context(tc.tile_pool(name="outsb", bufs=2))
            ps_agg = ctx.enter_context(tc.tile_pool(name="ps_agg", bufs=2, space="PSUM"))
            ps_out = ctx.enter_context(tc.tile_pool(name="ps_out", bufs=2, space="PSUM"))

            idx = const.tile(list(idx_t.shape), i16)
            drel = const.tile(list(drel_t.shape), gdt)
            wgt = const.tile(list(wgt_t.shape), gdt)
            iota = const.tile([128, 128, gmx], gdt)
            wmat = const.tile([D, D], gdt)
            bcol = const.tile([D, 1], f32)
            agg_all = const.tile([128, npad], gdt, tag="agg_all")

            for sb, dr in ((idx, idx_t), (drel, drel_t), (wgt, wgt_t),
                           (iota, iota_t), (wmat, w_t), (bcol, b_t)):
                nc.sync.dma_start(sb[:], dr[:])

            h_A = h_t[0:IMG_A_ROWS, :]
            h_B = h_t[IMG_B_BASE:N_NODES, :]

            col = 0    # idx column cursor (units of 8 cols per group)
            gbase = 0  # group column cursor
            done_slots = 0
            next_t0 = 0
            CH = 512

            def emit_transform(t0, n):
                po = ps_out.tile([128, CH], f32, tag="psout")
                nc.tensor.matmul(
                    po[:, :n], wmat[:], agg_all[:, t0 : t0 + n],
                    start=True, stop=True,
                )
                ob = opool.tile([128, CH], f32, tag="outsb")
                nc.scalar.add(ob[:, :n], po[:, :n], bcol[:])
                nc.sync.dma_start(out_t[:, t0 : t0 + n], ob[:, :n])

            prev = None  # (group_tiles, s, cross_col0, cA, cB, gA)
            for slot in range(nwin):
                gA, gB, g = int(gA_slot[slot]), int(gB_slot[slot]), int(g_slot[slot])
                dA, dB = int(dupA_g[slot]), int(dupB_g[slot])
                cA, cB = int(crsA_g[slot]), int(crsB_g[slot])
                ncols = g + dA + dB + cA + cB
                gtiles = []
                group_tiles = []  # flat (tile, j) per group, A then B
                for img, gimg in ((h_A, gA), (h_B, gB)):
                    for (c0, k) in _chunks(gimg):
                        gt = gpool.tile([128, k, 128], gdt, tag="g")
                        nc.gpsimd.dma_gather(
                            gt[:], img, idx[:, col : col + k * 8],
                            num_idxs=k * 128, num_idxs_reg=k * 128, elem_size=D,
                        )
                        col += k * 8
                        gtiles.append((gt, k))
                        for j in range(k):
                            group_tiles.append((gt, j))

                # S transposed [p, dst, group]: innermost AP step is 1 on
                # every operand (drel/wgt broadcast at the ROW level only),
                # which qualifies for the DVE 2x packed perf mode; the
                # iota_big constant supplies value n for all of row n.
                s = spool.tile([128, 128, ncols], gdt, tag="sel")
                sh = (128, 128, ncols)
                if os.environ.get("GCN_SPROBE", "0") == "1":
                    # perf probe: no DVE S-build (output is wrong)
                    nc.vector.memset(s[:], 0)
                else:
                    nc.vector.tensor_tensor(
                        s[:], iota[:, :, 0:ncols],
                        drel[:, None, gbase : gbase + ncols].broadcast_to(sh),
                        mybir.AluOpType.is_equal,
                    )
                    nc.vector.tensor_tensor(
                        s[:], s[:],
                        wgt[:, None, gbase : gbase + ncols].broadcast_to(sh),
                        mybir.AluOpType.mult,
                    )

                # matmul plan: group j -> primary S col j; dup-prefix groups
                # also get a secondary matmul (same stationary G).
                plan = []
                for j in range(g):
                    cols_j = [j]
                    if j < dA:
                        cols_j.append(g + j)
                    elif gA <= j < gA + dB:
                        cols_j.append(g + dA + (j - gA))
                    plan.append(cols_j)
                n_mm = sum(len(cj) for cj in plan)
                borrowed = []
                if prev is not None:
                    pgt, ps_, pc0, pcA, pcB, pgA = prev
                    for j in range(pcA):
                        borrowed.append((pgt[j], ps_, pc0 + j))
                    for j in range(pcB):
                        borrowed.append((pgt[pgA + j], ps_, pc0 + pcA + j))
                n_mm += len(borrowed)

                psum = ps_agg.tile([128, 128], f32, tag="psagg")
                mm = 0
                for ((bgt, bj), bs, bcol_) in borrowed:
                    nc.tensor.matmul(
                        psum[:], bgt[:, bj, :], bs[:, :, bcol_],
                        start=(mm == 0), stop=(mm == n_mm - 1),
                    )
                    mm += 1
                gi = 0
                for (gt, k) in gtiles:
                    for j in range(k):
                        for scol in plan[gi]:
                            nc.tensor.matmul(
                                psum[:], gt[:, j, :], s[:, :, scol],
                                start=(mm == 0), stop=(mm == n_mm - 1),
                            )
                            mm += 1
                        gi += 1
                nc.scalar.copy(agg_all[:, slot * WIN : (slot + 1) * WIN], psum[:])
                prev = (group_tiles, s, g + dA + dB, cA, cB, gA)
                gbase += ncols
                done_slots += 1
                # transform any complete 512-col chunk whose slots are done
                while done_slots * WIN >= next_t0 + CH:
                    emit_transform(next_t0, CH)
                    next_t0 += CH

            while next_t0 < npad:
                n = min(CH, npad - next_t0)
                emit_transform(next_t0, n)
                next_t0 += n

    nc.compile()
    return nc


def kernel(H, edge_src, edge_dst, edge_weight, W, b):
    global LAST_EXEC_NS, LAST_RESULTS
    from concourse import bass_utils

    H = np.asarray(H, dtype=np.float32)
    edge_src = np.asarray(edge_src, dtype=np.int32)
    edge_dst = np.asarray(edge_dst, dtype=np.int32)
    edge_weight = np.asarray(edge_weight, dtype=np.float32)
    W = np.asarray(W, dtype=np.float32)
    b = np.asarray(b, dtype=np.float32)

    np_g = np.float16 if GDTYPE == "f16" else np.float32
    (packets, order, nwin, g_slot, gA_slot, gB_slot, dupA_g, dupB_g,
     crsA_g, crsB_g) = _prep(edge_src, edge_dst, edge_weight)

    h_src = np.ascontiguousarray(H.astype(np_g))
    gmx = int((g_slot + dupA_g + dupB_g + crsA_g + crsB_g).max())
    iota = np.tile(np.repeat(np.arange(128, dtype=np_g), gmx), (128, 1))
    wmat = np.ascontiguousarray(W.astype(np_g))
    bcol = np.ascontiguousarray(b.astype(np.float32).reshape(D, 1))
    in_maps = []
    idx_cols = None
    for c in range(N_CORES):
        idx, drel, wgt = _pack_core(
            packets[c], nwin, g_slot, gA_slot, gB_slot, dupA_g, dupB_g,
            crsA_g, crsB_g, np_g,
        )
        idx_cols = idx.shape[1]
        in_maps.append({
            "h_src": h_src, "idx": idx, "drel": drel, "wgt": wgt,
            "iota": iota, "wmat": wmat, "bcol": bcol,
        })

    nc = _build_program(nwin, g_slot, gA_slot, gB_slot, dupA_g, dupB_g,
                        crsA_g, crsB_g, idx_cols)

    if os.environ.get("GCN_SIM", "0") == "1":
        from concourse.bass_interp import CoreSim

        out = np.empty((N_NODES, D), np.float32)
        for c in range(N_CORES):
            sim = CoreSim(nc)
            for k2, v2 in in_maps[c].items():
                sim.tensor(k2)[:] = v2
            sim.simulate()
            outT = np.array(sim.tensor("outT"))
            # slot i columns -> window order[c][i]
            for slot in range(nwin):
                wi = int(order[c][slot])
                w0, w1 = wi * WIN, min((wi + 1) * WIN, NPW)
                out[c * NPW + w0 : c * NPW + w1, :] = (
                    outT[:, slot * WIN : slot * WIN + (w1 - w0)].T)
        return out

    trace = os.environ.get("GCN_TRACE", "0") == "1"
    kw = {}
    if trace:
        import shutil
        td = "/tmp/gcn_ntff"
        shutil.rmtree(td, ignore_errors=True)
        os.makedirs(td, exist_ok=True)
        kw["tmpdir"] = td
    import time as _time
    last_err = None
    for backoff in (15, 45, 90, 0):
        try:
            res = bass_utils.run_bass_kernel_spmd(
                nc, in_maps, core_ids=list(range(N_CORES)), trace=trace, **kw
            )
            break
        except Exception as e:
            last_err = e
            if backoff:
                _time.sleep(backoff)
    else:
        raise last_err
    LAST_EXEC_NS = res.exec_time_ns
    LAST_RESULTS = res

    out = np.empty((N_NODES, D), np.float32)
    for c in range(N_CORES):
        outT = res.results[c]["outT"]
        for slot in range(nwin):
            wi = int(order[c][slot])
            w0, w1 = wi * WIN, min((wi + 1) * WIN, NPW)
            out[c * NPW + w0 : c * NPW + w1, :] = (
                outT[:, slot * WIN : slot * WIN + (w1 - w0)].T)
    return out
